# revision 5
# baseline (speedup 1.0000x reference)
"""Cross-attention (ALiBi) Trainium2 kernel, v2: per-slot banding + pipelined
normalization.

Sharding: 8 cores = 2 batches x 4 head-slot-groups. Head dealing is by ALiBi
window so every core gets the same banded tile pattern (SPMD-safe):
  core (b, g) slots = [12+g, 8+g, 4+g, g]   (slot windows [full, 448, 112, 28])
Per-slot j-tile lists per 512-i-chunk (validated banding err ~1e-5):
  slot0: all 16;  slot1: [(0,7),(0,11),(4,15),(8,15)];
  slots2,3: [(0,4),(3,8),(7,12),(11,15)]

Per core: q/k/v projections (kt-outer, 8 psum banks, bias folded into DVE
evacuation), banded attention with multiplicative ALiBi (Toeplitz strip),
softmax denom via ones-column in v, reciprocal_approx_fast + deferred norm
muls, partial output projection (row-sharded Wo); host sums partials + bo.

Layouts (per core):
  qT, kT  [128, 2 pr, 2048 n] bf16   slot s: pr=s//2, partitions (s%2)*64..
  v       [128, 16 jt, 65*4] bf16    slot s cols 65s..65s+64, ones col 65s+64
  scoresT [j, i] per head; ALiBi multiplicative: p = exp(s/8) * estrip,
  estrip sliced from [128, 3968] per head, u0 = 1920 - 128*jt + 512*ic.
PSUM (8 banks, shared tags between proj and attention phases):
  s2 [128,2,512] x2, o2 [128,2,512] x1, op [128,512] x2
"""

import sys
import numpy as np
import ml_dtypes
from contextlib import ExitStack

if "/opt/trn_rl_repo" not in sys.path:
    sys.path.insert(0, "/opt/trn_rl_repo")

B, N, E, H, D = 2, 2048, 1024, 16, 64
HPC = 4            # heads per core
ES = HPC * D       # 256 e'-columns per core
NCORES = 8
KT = E // 128      # 8 contraction tiles for projections
NT = N // 128      # 16 n/j tiles
NC512 = N // 512   # 4 chunks of 512
USTRIP = 3968

BF16 = ml_dtypes.bfloat16

# banded j-tile ranges [first, last] per ic for each slot
JTS1 = [(0, 7), (0, 11), (4, 15), (8, 15)]    # slot 1 (w=448)
JTS23 = [(0, 4), (3, 8), (7, 12), (11, 15)]   # slots 2,3 (w=112)
RECIP_FAST = False


def _jts(slot, ic):
    if slot == 0:
        return (0, NT - 1)
    if slot == 1:
        return JTS1[ic]
    return JTS23[ic]


# slot -> head for group g: heads [12+g, 8+g, 4+g, g]
def _slot_heads(g):
    return [12 + g, 8 + g, 4 + g, g]


_cache: dict = {}


def _alibi_slopes():
    return np.array([2.0 ** (-8.0 * (h + 1) / H) for h in range(H)], dtype=np.float64)


def _estrips():
    """[4 groups][4 slots, 128, 3968] bf16: strip[p, u] = exp(-slope*|p+1920-u|)."""
    if "estrips" in _cache:
        return _cache["estrips"]
    slopes = _alibi_slopes()
    au = np.abs(np.arange(128)[:, None] + 1920 - np.arange(USTRIP)[None, :]).astype(np.float64)
    groups = []
    for g in range(4):
        heads = []
        for h in _slot_heads(g):
            heads.append(np.exp(-slopes[h] * au))
        groups.append(np.stack(heads).astype(BF16))
    _cache["estrips"] = groups
    return groups


def _build():
    import concourse.bass as bass
    import concourse.mybir as mybir
    import concourse.tile as tile
    from concourse import bacc

    fp32 = mybir.dt.float32
    bf16 = mybir.dt.bfloat16
    AF = mybir.ActivationFunctionType

    nc = bacc.Bacc("TRN2", target_bir_lowering=False, debug=False)

    qtt = nc.dram_tensor("qtt", [E, N], bf16, kind="ExternalInput").ap()
    kvt = nc.dram_tensor("kvt", [E, N], bf16, kind="ExternalInput").ap()
    # weights pre-arranged host-side to SBUF layout (plain 2D DMAs)
    wq = nc.dram_tensor("wq", [128, KT * ES], bf16, kind="ExternalInput").ap()
    wk = nc.dram_tensor("wk", [128, KT * ES], bf16, kind="ExternalInput").ap()
    wv = nc.dram_tensor("wv", [128, KT * ES], bf16, kind="ExternalInput").ap()
    wo = nc.dram_tensor("wo", [128, 2 * E], bf16, kind="ExternalInput").ap()
    # bqk cols: [bq_t0, bq_t1, bk_t0, bk_t1] as per-partition scalars
    bqk = nc.dram_tensor("bqk", [128, 4], fp32, kind="ExternalInput").ap()
    # bv broadcast across partitions [128, ES]
    bvb = nc.dram_tensor("bvb", [128, ES], bf16, kind="ExternalInput").ap()
    estrip = nc.dram_tensor("estrip", [128, HPC * USTRIP], bf16, kind="ExternalInput").ap()
    out = nc.dram_tensor("out", [N, E], bf16, kind="ExternalOutput").ap()

    with tile.TileContext(nc) as tc, ExitStack() as ctx:
        consts = ctx.enter_context(tc.tile_pool(name="consts", bufs=1))
        big = ctx.enter_context(tc.tile_pool(name="big", bufs=1))
        acts = ctx.enter_context(tc.tile_pool(name="acts", bufs=1))
        ptpool = ctx.enter_context(tc.tile_pool(name="ptpool", bufs=6))
        smalls = ctx.enter_context(tc.tile_pool(name="smalls", bufs=2))
        outsb = ctx.enter_context(tc.tile_pool(name="outsb", bufs=3))
        psum = ctx.enter_context(tc.tile_pool(name="psum", bufs=2, space="PSUM"))

        # ---- DMA: biases + weights first (per-kt so subtile deps release
        # early), inputs kt-interleaved across both queues, then wk / estrip /
        # wo in consumption order ----
        bqk_sb = consts.tile([128, 4], fp32)
        nc.sync.dma_start(bqk_sb[:], bqk)
        bvb_sb = consts.tile([128, ES], bf16)
        nc.scalar.dma_start(bvb_sb[:], bvb)

        wq_sb = consts.tile([128, KT, ES], bf16)
        wk_sb = consts.tile([128, KT, ES], bf16)
        wv_sb = consts.tile([128, KT, ES], bf16)
        for k in range(KT):
            nc.sync.dma_start(wq_sb[:, k, :], wq[:, k * ES:(k + 1) * ES])
            nc.scalar.dma_start(wv_sb[:, k, :], wv[:, k * ES:(k + 1) * ES])

        qtt_sb = big.tile([128, KT, N], bf16)
        kvt_sb = big.tile([128, KT, N], bf16)
        # q inputs first (q phase), kvt next (v phase), wk before kvt tail
        for k in range(KT):
            qq = nc.sync if k % 2 == 0 else nc.scalar
            qq.dma_start(qtt_sb[:, k, :], qtt[k * 128:(k + 1) * 128, :])
        for k in range(KT):
            nc.sync.dma_start(wk_sb[:, k, :], wk[:, k * ES:(k + 1) * ES])
        for k in range(KT):
            kq = nc.scalar if k % 2 == 0 else nc.sync
            kq.dma_start(kvt_sb[:, k, :], kvt[k * 128:(k + 1) * 128, :])

        es_sb = consts.tile([128, HPC, USTRIP], bf16)
        for s in range(HPC):
            (nc.sync if s % 2 == 0 else nc.scalar).dma_start(
                es_sb[:, s, :], estrip[:, s * USTRIP:(s + 1) * USTRIP])
        wo_sb = consts.tile([128, 2, E], bf16)
        nc.scalar.dma_start(wo_sb[:], wo.rearrange("p (t e) -> p t e", t=2))

        qT_sb = acts.tile([128, 2, N], bf16)
        kT_sb = acts.tile([128, 2, N], bf16)
        v_sb = acts.tile([128, NT, 65 * HPC], bf16)
        oT_sb = acts.tile([128, 2, N], bf16)

        # ones columns of v (softmax denominator trick)
        nc.vector.memset(
            v_sb[:, :, :].rearrange("p t (h c) -> p t h c", c=65)[:, :, :, 64:65], 1.0)

        TC8 = [(t, c) for t in range(2) for c in range(NC512)]

        # ---- q/k projections: kt-outer, 8 chunk accumulators over 8 banks ----
        def proj_qk(w_sb, dst, bcol):
            p1 = psum.tile([128, 2, 512], fp32, tag="s2", name="p1")
            p2 = psum.tile([128, 2, 512], fp32, tag="s2", name="p2")
            p3 = psum.tile([128, 2, 512], fp32, tag="o2", name="p3", bufs=1)
            p4 = psum.tile([128, 512], fp32, tag="op", name="p4")
            p5 = psum.tile([128, 512], fp32, tag="op", name="p5")
            slots8 = [p1[:, 0, :], p1[:, 1, :], p2[:, 0, :], p2[:, 1, :],
                      p3[:, 0, :], p3[:, 1, :], p4[:], p5[:]]
            for k in range(KT):
                for idx, (t, c) in enumerate(TC8):
                    nc.tensor.matmul(
                        slots8[idx],
                        w_sb[:, k, t * 128:(t + 1) * 128],
                        qtt_sb[:, k, c * 512:(c + 1) * 512] if dst is qT_sb
                        else kvt_sb[:, k, c * 512:(c + 1) * 512],
                        start=(k == 0), stop=(k == KT - 1),
                    )
            for idx, (t, c) in enumerate(TC8):
                nc.vector.tensor_scalar_add(
                    dst[:, t, c * 512:(c + 1) * 512], slots8[idx],
                    bqk_sb[:, bcol + t:bcol + t + 1])

        proj_qk(wq_sb, qT_sb, 0)

        # ---- v projection: kt-outer, 16 half-bank accumulators ----
        vp1 = psum.tile([128, 4, 256], fp32, tag="s2", name="vp1")
        vp2 = psum.tile([128, 4, 256], fp32, tag="s2", name="vp2")
        vp3 = psum.tile([128, 4, 256], fp32, tag="o2", name="vp3", bufs=1)
        vp4 = psum.tile([128, 2, 256], fp32, tag="op", name="vp4")
        vp5 = psum.tile([128, 2, 256], fp32, tag="op", name="vp5")
        vslots = ([vp1[:, i, :] for i in range(4)] + [vp2[:, i, :] for i in range(4)]
                  + [vp3[:, i, :] for i in range(4)]
                  + [vp4[:, i, :] for i in range(2)] + [vp5[:, i, :] for i in range(2)])
        # bank-alternating emission order within each kt. Two jts share each
        # psum bank; start=True clears has_written for the WHOLE bank, so only
        # the first jt of each bank issues start=True (the second jt's k==0
        # matmul overwrites-where-unset after that bank-wide clear).
        VORD = [0, 2, 1, 3, 4, 6, 5, 7, 8, 10, 9, 11, 12, 14, 13, 15]
        BANK_FIRST = {0, 2, 4, 6, 8, 10, 12, 14}
        for k in range(KT):
            for jt in VORD:
                nc.tensor.matmul(
                    vslots[jt],
                    kvt_sb[:, k, jt * 128:(jt + 1) * 128],
                    wv_sb[:, k, :],
                    start=(k == 0 and jt in BANK_FIRST), stop=(k == KT - 1),
                    skip_group_check=(jt not in BANK_FIRST),
                )
        for jt in range(NT):
            nc.vector.tensor_add(
                v_sb[:, jt, :].rearrange("p (h c) -> p h c", c=65)[:, :, 0:64],
                vslots[jt].rearrange("p (h c) -> p h c", c=64),
                bvb_sb.rearrange("p (h c) -> p h c", c=64),
            )

        proj_qk(wk_sb, kT_sb, 2)

        # ---- attention: ic outer, pair-groups (slots 0,1) and (2,3) ----
        dq = []          # deferred closures: norm muls, outproj groups
        outproj_ring = []

        def emit_outproj(nt, ec, alt):
            op_ps = psum.tile([128, 512], fp32, tag="op", name="op_ps")
            for t in range(2):
                nc.tensor.matmul(
                    op_ps[:],
                    oT_sb[:, t, nt * 128:(nt + 1) * 128],
                    wo_sb[:, t, ec * 512:(ec + 1) * 512],
                    start=(t == 0), stop=(t == 1),
                )
            o_sb = outsb.tile([128, 512], bf16, name="o_sb")
            if alt:
                nc.scalar.copy(o_sb[:], op_ps[:])
            else:
                nc.vector.tensor_copy(o_sb[:], op_ps[:])
            nc.sync.dma_start(
                out[nt * 128:(nt + 1) * 128, ec * 512:(ec + 1) * 512], o_sb[:])

        for ic in range(NC512):
            isl = slice(ic * 512, (ic + 1) * 512)
            for pg in range(2):
                sa, sb = 2 * pg, 2 * pg + 1
                a_lo, a_hi = _jts(sa, ic)
                b_lo, b_hi = _jts(sb, ic)
                o2 = psum.tile([65, 2, 512], fp32, tag="o2", name="o2", bufs=1)

                def emit_pv(jt, pt, dual):
                    nc.tensor.matmul(
                        o2[:, 0, :], v_sb[:, jt, sa * 65:sa * 65 + 65], pt[:, 0, :],
                        start=(jt == a_lo), stop=(jt == a_hi),
                    )
                    if dual:
                        nc.tensor.matmul(
                            o2[:, 1, :], v_sb[:, jt, sb * 65:sb * 65 + 65], pt[:, 1, :],
                            start=(jt == b_lo), stop=(jt == b_hi),
                        )

                prev = None
                for step, jt in enumerate(range(a_lo, a_hi + 1)):
                    dual = b_lo <= jt <= b_hi
                    s2 = psum.tile([128, 2, 512], fp32, tag="s2", name="s2")
                    nc.tensor.matmul(
                        s2[:, 0, :],
                        kT_sb[0:64, pg, jt * 128:(jt + 1) * 128],
                        qT_sb[0:64, pg, isl],
                        start=True, stop=True, tile_position=(0, 0),
                    )
                    if dual:
                        nc.tensor.matmul(
                            s2[:, 1, :],
                            kT_sb[64:128, pg, jt * 128:(jt + 1) * 128],
                            qT_sb[64:128, pg, isl],
                            start=True, stop=True, tile_position=(64, 0),
                        )
                    pt = ptpool.tile([128, 2, 512], bf16, tag="pt", name="pt")
                    u0 = 1920 - 128 * jt + 512 * ic
                    if dual:
                        nc.scalar.activation(pt[:], s2[:], AF.Exp, scale=0.125)
                        nc.vector.tensor_mul(
                            pt[:], pt[:], es_sb[:, sa:sa + 2, u0:u0 + 512])
                    else:
                        nc.scalar.activation(
                            pt[:, 0, :], s2[:, 0, :], AF.Exp, scale=0.125)
                        nc.vector.tensor_mul(
                            pt[:, 0, :], pt[:, 0, :], es_sb[:, sa, u0:u0 + 512])
                    if step >= 2 and dq:
                        dq.pop(0)()
                    if prev is not None:
                        emit_pv(*prev)
                    prev = (jt, pt, dual)
                emit_pv(*prev)

                # evacuate o2 promptly (frees psum), reciprocal, defer norm muls
                o_un = smalls.tile([65, 2, 512], fp32, tag="o_un", name="o_un")
                nc.vector.tensor_copy(o_un[:], o2[:])
                recip = smalls.tile([1, 2, 512], fp32, tag="recip", name="recip")
                if RECIP_FAST:
                    nc.vector.reciprocal_approx_fast(recip[:], o_un[64:65, :, :])
                else:
                    nc.vector.reciprocal(recip[:], o_un[64:65, :, :])
                rb = smalls.tile([64, 2, 512], fp32, tag="rb", name="rb")
                nc.gpsimd.partition_broadcast(rb[:], recip[:])

                def norm(pg=pg, isl=isl, o_un=o_un, rb=rb):
                    nc.vector.tensor_mul(
                        oT_sb[0:64, pg, isl], o_un[0:64, 0, :], rb[:, 0, :])
                    nc.vector.tensor_mul(
                        oT_sb[64:128, pg, isl], o_un[0:64, 1, :], rb[:, 1, :])
                dq.append(norm)
                if pg == 1:
                    for i, (nt, ec) in enumerate(
                            (nt, ec) for nt in range(4 * ic, 4 * ic + 4)
                            for ec in range(2)):
                        dq.append(lambda nt=nt, ec=ec, i=i: emit_outproj(nt, ec, i % 2))
        while dq:
            dq.pop(0)()

    nc.compile()
    return nc


def _get_nc():
    if "nc" not in _cache:
        _cache["nc"] = _build()
    return _cache["nc"]


def _warr(w):
    """[E, ES] -> [128, KT*ES] sbuf layout: row p = concat_k w[k*128+p, :]."""
    return np.ascontiguousarray(
        w.reshape(KT, 128, ES).transpose(1, 0, 2).reshape(128, KT * ES)
    ).astype(BF16)


def _in_maps(query, kv, Wq, bq, Wkv, bkv, Wo, bo):
    strips = _estrips()
    qT = [np.ascontiguousarray(query[b].T).astype(BF16) for b in range(B)]
    kvT = [np.ascontiguousarray(kv[b].T).astype(BF16) for b in range(B)]
    maps = []
    for c in range(NCORES):
        b, g = c // 4, c % 4
        heads = _slot_heads(g)
        cols = np.concatenate([np.arange(h * D, (h + 1) * D) for h in heads])
        wo_arr = np.ascontiguousarray(
            Wo[cols, :].reshape(2, 128, E).transpose(1, 0, 2).reshape(128, 2 * E)
        ).astype(BF16)
        es_arr = np.ascontiguousarray(
            strips[g].transpose(1, 0, 2).reshape(128, HPC * USTRIP))
        bq_c = np.asarray(bq)[cols].astype(np.float32)        # [256]
        bk_c = np.asarray(bkv)[:E][cols].astype(np.float32)
        bv_c = np.asarray(bkv)[E:][cols].astype(BF16)
        bqk_arr = np.stack(
            [bq_c[0:128], bq_c[128:256], bk_c[0:128], bk_c[128:256]], axis=1)
        bvb_arr = np.ascontiguousarray(
            np.broadcast_to(bv_c[None, :], (128, ES)))
        maps.append({
            "qtt": qT[b],
            "kvt": kvT[b],
            "wq": _warr(Wq[:, cols]),
            "wk": _warr(Wkv[:, :E][:, cols]),
            "wv": _warr(Wkv[:, E:][:, cols]),
            "wo": wo_arr,
            "bqk": np.ascontiguousarray(bqk_arr),
            "bvb": bvb_arr,
            "estrip": es_arr,
        })
    return maps


def kernel(query, kv, Wq, bq, Wkv, bkv, Wo, bo, _collect=None):
    from concourse import bass_utils

    query = np.asarray(query, dtype=np.float32)
    kv = np.asarray(kv, dtype=np.float32)
    nc = _get_nc()
    maps = _in_maps(query, kv, np.asarray(Wq), np.asarray(bq), np.asarray(Wkv),
                    np.asarray(bkv), np.asarray(Wo), np.asarray(bo))
    res = bass_utils.run_bass_kernel_spmd(
        nc, maps, core_ids=list(range(NCORES)),
        **(_collect or {}),
    )
    if _collect is not None:
        _cache["last_results"] = res
    outp = np.zeros((B, N, E), dtype=np.float32)
    for c in range(NCORES):
        outp[c // 4] += res.results[c]["out"].astype(np.float32)
    outp += np.asarray(bo, dtype=np.float32)
    return outp


# revision 14
# speedup vs baseline: 1.3739x; 1.3739x over previous
"""Cross-attention (ALiBi) Trainium2 kernel, v2: per-slot banding + pipelined
normalization.

Sharding: 8 cores = 2 batches x 4 head-slot-groups. Head dealing is by ALiBi
window so every core gets the same banded tile pattern (SPMD-safe):
  core (b, g) slots = [12+g, 8+g, 4+g, g]   (slot windows [full, 448, 112, 28])
Per-slot j-tile lists per 512-i-chunk (validated banding err ~1e-5):
  slot0: all 16;  slot1: [(0,7),(0,11),(4,15),(8,15)];
  slots2,3: [(0,4),(3,8),(7,12),(11,15)]

Per core: q/k/v projections (kt-outer, 8 psum banks, bias folded into DVE
evacuation), banded attention with multiplicative ALiBi (Toeplitz strip),
softmax denom via ones-column in v, reciprocal_approx_fast + deferred norm
muls, partial output projection (row-sharded Wo); host sums partials + bo.

Layouts (per core):
  qT, kT  [128, 2 pr, 2048 n] bf16   slot s: pr=s//2, partitions (s%2)*64..
  v       [128, 16 jt, 65*4] bf16    slot s cols 65s..65s+64, ones col 65s+64
  scoresT [j, i] per head; ALiBi multiplicative: p = exp(s/8) * estrip,
  estrip sliced from [128, 3968] per head, u0 = 1920 - 128*jt + 512*ic.
PSUM (8 banks, shared tags between proj and attention phases):
  s2 [128,2,512] x2, o2 [128,2,512] x1, op [128,512] x2
"""

import sys
import numpy as np
import ml_dtypes
from contextlib import ExitStack

if "/opt/trn_rl_repo" not in sys.path:
    sys.path.insert(0, "/opt/trn_rl_repo")

B, N, E, H, D = 2, 2048, 1024, 16, 64
HPC = 4            # heads per core
ES = HPC * D       # 256 e'-columns per core
NCORES = 8
KT = E // 128      # 8 contraction tiles for projections
NT = N // 128      # 16 n/j tiles
NC512 = N // 512   # 4 chunks of 512
USTRIP = 3968

BF16 = ml_dtypes.bfloat16

# banded j-tile ranges [first, last] per ic for each slot
JTS1 = [(0, 7), (0, 11), (4, 15), (8, 15)]    # slot 1 (w=448)
JTS23 = [(0, 4), (3, 8), (7, 12), (11, 15)]   # slots 2,3 (w=112)
RECIP_FAST = True


def _jts(slot, ic):
    if slot == 0:
        return (0, NT - 1)
    if slot == 1:
        return JTS1[ic]
    return JTS23[ic]


# slot -> head for group g: heads [12+g, 8+g, 4+g, g]
def _slot_heads(g):
    return [12 + g, 8 + g, 4 + g, g]


_cache: dict = {}


def _alibi_slopes():
    return np.array([2.0 ** (-8.0 * (h + 1) / H) for h in range(H)], dtype=np.float64)


def _estrips():
    """[4 groups][4 slots, 128, 3968] bf16: strip[p, u] = exp(-slope*|p+1920-u|)."""
    if "estrips" in _cache:
        return _cache["estrips"]
    slopes = _alibi_slopes()
    au = np.abs(np.arange(128)[:, None] + 1920 - np.arange(USTRIP)[None, :]).astype(np.float64)
    groups = []
    for g in range(4):
        heads = []
        for h in _slot_heads(g):
            heads.append(np.exp(-slopes[h] * au))
        groups.append(np.stack(heads).astype(BF16))
    _cache["estrips"] = groups
    return groups


def _build():
    import concourse.bass as bass
    import concourse.mybir as mybir
    import concourse.tile as tile
    from concourse import bacc

    fp32 = mybir.dt.float32
    bf16 = mybir.dt.bfloat16
    AF = mybir.ActivationFunctionType

    nc = bacc.Bacc("TRN2", target_bir_lowering=False, debug=False)

    qtt = nc.dram_tensor("qtt", [E, N], bf16, kind="ExternalInput").ap()
    kvt = nc.dram_tensor("kvt", [E, N], bf16, kind="ExternalInput").ap()
    # weights pre-arranged host-side to SBUF layout (plain 2D DMAs)
    wq = nc.dram_tensor("wq", [128, KT * ES], bf16, kind="ExternalInput").ap()
    wk = nc.dram_tensor("wk", [128, KT * ES], bf16, kind="ExternalInput").ap()
    wv = nc.dram_tensor("wv", [128, KT * ES], bf16, kind="ExternalInput").ap()
    wo = nc.dram_tensor("wo", [128, 2 * E], bf16, kind="ExternalInput").ap()
    # bqk cols: [bq_t0, bq_t1, bk_t0, bk_t1] as per-partition scalars
    bqk = nc.dram_tensor("bqk", [128, 4], fp32, kind="ExternalInput").ap()
    # bv broadcast across partitions [128, ES]
    bvb = nc.dram_tensor("bvb", [128, ES], bf16, kind="ExternalInput").ap()
    estrip = nc.dram_tensor("estrip", [128, HPC * USTRIP], bf16, kind="ExternalInput").ap()
    out = nc.dram_tensor("out", [N, E], bf16, kind="ExternalOutput").ap()

    with tile.TileContext(nc) as tc, ExitStack() as ctx:
        consts = ctx.enter_context(tc.tile_pool(name="consts", bufs=1))
        big = ctx.enter_context(tc.tile_pool(name="big", bufs=1))
        acts = ctx.enter_context(tc.tile_pool(name="acts", bufs=1))
        ptpool = ctx.enter_context(tc.tile_pool(name="ptpool", bufs=6))
        smalls = ctx.enter_context(tc.tile_pool(name="smalls", bufs=2))
        outsb = ctx.enter_context(tc.tile_pool(name="outsb", bufs=3))
        psum = ctx.enter_context(tc.tile_pool(name="psum", bufs=2, space="PSUM"))

        # ---- DMA: biases + weights first (per-kt so subtile deps release
        # early), inputs kt-interleaved across both queues, then wk / estrip /
        # wo in consumption order ----
        bqk_sb = consts.tile([128, 4], fp32)
        nc.sync.dma_start(bqk_sb[:], bqk)
        bvb_sb = consts.tile([128, ES], bf16)
        nc.scalar.dma_start(bvb_sb[:], bvb)

        wq_sb = consts.tile([128, KT, ES], bf16)
        wk_sb = consts.tile([128, KT, ES], bf16)
        wv_sb = consts.tile([128, KT, ES], bf16)
        for k in range(KT):
            nc.sync.dma_start(wq_sb[:, k, :], wq[:, k * ES:(k + 1) * ES])
            nc.scalar.dma_start(wv_sb[:, k, :], wv[:, k * ES:(k + 1) * ES])

        qtt_sb = big.tile([128, KT, N], bf16)
        kvt_sb = big.tile([128, KT, N], bf16)
        # q inputs first (q phase), kvt next (v phase), wk before kvt tail
        for k in range(KT):
            qq = nc.sync if k % 2 == 0 else nc.scalar
            qq.dma_start(qtt_sb[:, k, :], qtt[k * 128:(k + 1) * 128, :])
        for k in range(KT):
            nc.sync.dma_start(wk_sb[:, k, :], wk[:, k * ES:(k + 1) * ES])
        for k in range(KT):
            kq = nc.scalar if k % 2 == 0 else nc.sync
            kq.dma_start(kvt_sb[:, k, :], kvt[k * 128:(k + 1) * 128, :])

        es_sb = consts.tile([128, HPC, USTRIP], bf16)
        for s in range(HPC):
            (nc.sync if s % 2 == 0 else nc.scalar).dma_start(
                es_sb[:, s, :], estrip[:, s * USTRIP:(s + 1) * USTRIP])
        wo_sb = consts.tile([128, 2, E], bf16)
        nc.scalar.dma_start(wo_sb[:], wo.rearrange("p (t e) -> p t e", t=2))

        qT_sb = acts.tile([128, 2, N], bf16)
        kT_sb = acts.tile([128, 2, N], bf16)
        v_sb = acts.tile([128, NT, 65 * HPC], bf16)
        oT_sb = acts.tile([128, 2, N], bf16)

        # ones columns of v (softmax denominator trick)
        nc.vector.memset(
            v_sb[:, :, :].rearrange("p t (h c) -> p t h c", c=65)[:, :, :, 64:65], 1.0)

        TC8 = [(t, c) for t in range(2) for c in range(NC512)]

        # ---- q/k projections: kt-outer, 8 chunk accumulators over 8 banks ----
        def proj_qk(w_sb, dst, bcol):
            p1 = psum.tile([128, 2, 512], fp32, tag="s2", name="p1")
            p2 = psum.tile([128, 2, 512], fp32, tag="s2", name="p2")
            p3 = psum.tile([128, 2, 512], fp32, tag="o2", name="p3", bufs=1)
            p4 = psum.tile([128, 512], fp32, tag="op", name="p4")
            p5 = psum.tile([128, 512], fp32, tag="op", name="p5")
            slots8 = [p1[:, 0, :], p1[:, 1, :], p2[:, 0, :], p2[:, 1, :],
                      p3[:, 0, :], p3[:, 1, :], p4[:], p5[:]]
            for k in range(KT):
                for idx, (t, c) in enumerate(TC8):
                    nc.tensor.matmul(
                        slots8[idx],
                        w_sb[:, k, t * 128:(t + 1) * 128],
                        qtt_sb[:, k, c * 512:(c + 1) * 512] if dst is qT_sb
                        else kvt_sb[:, k, c * 512:(c + 1) * 512],
                        start=(k == 0), stop=(k == KT - 1),
                    )
            for idx, (t, c) in enumerate(TC8):
                nc.vector.tensor_scalar_add(
                    dst[:, t, c * 512:(c + 1) * 512], slots8[idx],
                    bqk_sb[:, bcol + t:bcol + t + 1])

        proj_qk(wq_sb, qT_sb, 0)

        # ---- v projection: kt-outer, 16 half-bank accumulators ----
        vp1 = psum.tile([128, 4, 256], fp32, tag="s2", name="vp1")
        vp2 = psum.tile([128, 4, 256], fp32, tag="s2", name="vp2")
        vp3 = psum.tile([128, 4, 256], fp32, tag="o2", name="vp3", bufs=1)
        vp4 = psum.tile([128, 2, 256], fp32, tag="op", name="vp4")
        vp5 = psum.tile([128, 2, 256], fp32, tag="op", name="vp5")
        vslots = ([vp1[:, i, :] for i in range(4)] + [vp2[:, i, :] for i in range(4)]
                  + [vp3[:, i, :] for i in range(4)]
                  + [vp4[:, i, :] for i in range(2)] + [vp5[:, i, :] for i in range(2)])
        # bank-alternating emission order within each kt. Two jts share each
        # psum bank; start=True clears has_written for the WHOLE bank, so only
        # the first jt of each bank issues start=True (the second jt's k==0
        # matmul overwrites-where-unset after that bank-wide clear).
        VORD = [0, 2, 1, 3, 4, 6, 5, 7, 8, 10, 9, 11, 12, 14, 13, 15]
        BANK_FIRST = {0, 2, 4, 6, 8, 10, 12, 14}
        for k in range(KT):
            for jt in VORD:
                nc.tensor.matmul(
                    vslots[jt],
                    kvt_sb[:, k, jt * 128:(jt + 1) * 128],
                    wv_sb[:, k, :],
                    start=(k == 0 and jt in BANK_FIRST), stop=(k == KT - 1),
                    skip_group_check=(jt not in BANK_FIRST),
                )
        for jt in range(NT):
            nc.vector.tensor_add(
                v_sb[:, jt, :].rearrange("p (h c) -> p h c", c=65)[:, :, 0:64],
                vslots[jt].rearrange("p (h c) -> p h c", c=64),
                bvb_sb.rearrange("p (h c) -> p h c", c=64),
            )

        proj_qk(wk_sb, kT_sb, 2)

        # ---- attention: ic outer, pair-groups (slots 0,1) and (2,3) ----
        dq = []          # deferred closures: norm muls, outproj groups
        outproj_ring = []

        def emit_outproj(nt, ec, alt):
            op_ps = psum.tile([128, 512], fp32, tag="op", name="op_ps")
            for t in range(2):
                nc.tensor.matmul(
                    op_ps[:],
                    oT_sb[:, t, nt * 128:(nt + 1) * 128],
                    wo_sb[:, t, ec * 512:(ec + 1) * 512],
                    start=(t == 0), stop=(t == 1),
                )
            o_sb = outsb.tile([128, 512], bf16, name="o_sb")
            if alt:
                nc.scalar.copy(o_sb[:], op_ps[:])
            else:
                nc.vector.tensor_copy(o_sb[:], op_ps[:])
            nc.sync.dma_start(
                out[nt * 128:(nt + 1) * 128, ec * 512:(ec + 1) * 512], o_sb[:])

        for ic in range(NC512):
            isl = slice(ic * 512, (ic + 1) * 512)
            for pg in range(2):
                sa, sb = 2 * pg, 2 * pg + 1
                a_lo, a_hi = _jts(sa, ic)
                b_lo, b_hi = _jts(sb, ic)
                o2 = psum.tile([65, 2, 512], fp32, tag="o2", name="o2", bufs=1)

                def emit_pv(jt, pt, dual):
                    nc.tensor.matmul(
                        o2[:, 0, :], v_sb[:, jt, sa * 65:sa * 65 + 65], pt[:, 0, :],
                        start=(jt == a_lo), stop=(jt == a_hi),
                    )
                    if dual:
                        nc.tensor.matmul(
                            o2[:, 1, :], v_sb[:, jt, sb * 65:sb * 65 + 65], pt[:, 1, :],
                            start=(jt == b_lo), stop=(jt == b_hi),
                        )

                prev = None
                for step, jt in enumerate(range(a_lo, a_hi + 1)):
                    dual = b_lo <= jt <= b_hi
                    s2 = psum.tile([128, 2, 512], fp32, tag="s2", name="s2")
                    nc.tensor.matmul(
                        s2[:, 0, :],
                        kT_sb[0:64, pg, jt * 128:(jt + 1) * 128],
                        qT_sb[0:64, pg, isl],
                        start=True, stop=True, tile_position=(0, 0),
                    )
                    if dual:
                        nc.tensor.matmul(
                            s2[:, 1, :],
                            kT_sb[64:128, pg, jt * 128:(jt + 1) * 128],
                            qT_sb[64:128, pg, isl],
                            start=True, stop=True, tile_position=(64, 0),
                        )
                    pt = ptpool.tile([128, 2, 512], bf16, tag="pt", name="pt")
                    u0 = 1920 - 128 * jt + 512 * ic
                    if dual:
                        nc.scalar.activation(pt[:], s2[:], AF.Exp, scale=0.125)
                        nc.vector.tensor_mul(
                            pt[:], pt[:], es_sb[:, sa:sa + 2, u0:u0 + 512])
                    else:
                        nc.scalar.activation(
                            pt[:, 0, :], s2[:, 0, :], AF.Exp, scale=0.125)
                        nc.vector.tensor_mul(
                            pt[:, 0, :], pt[:, 0, :], es_sb[:, sa, u0:u0 + 512])
                    if step >= 2 and dq:
                        dq.pop(0)()
                    if prev is not None:
                        emit_pv(*prev)
                    prev = (jt, pt, dual)
                emit_pv(*prev)

                # evacuate o2 promptly (frees psum), reciprocal, defer norm muls.
                # The denominator row is copied to a partition-0 tile: the
                # custom reciprocal_approx_fast DVE op misbehaves at
                # base_partition 64.
                o_un = smalls.tile([65, 2, 512], fp32, tag="o_un", name="o_un")
                nc.vector.tensor_copy(o_un[0:64, :, :], o2[0:64, :, :])
                den0 = smalls.tile([1, 2, 512], fp32, tag="den0", name="den0")
                nc.vector.tensor_copy(den0[:], o2[64:65, :, :])
                recip = smalls.tile([1, 2, 512], fp32, tag="recip", name="recip")
                if RECIP_FAST:
                    nc.vector.reciprocal_approx_fast(recip[:], den0[:])
                else:
                    nc.vector.reciprocal(recip[:], den0[:])
                rb = smalls.tile([64, 2, 512], fp32, tag="rb", name="rb")
                nc.gpsimd.partition_broadcast(rb[:], recip[:])

                def norm(pg=pg, isl=isl, o_un=o_un, rb=rb):
                    nc.vector.tensor_mul(
                        oT_sb[0:64, pg, isl], o_un[0:64, 0, :], rb[:, 0, :])
                    nc.vector.tensor_mul(
                        oT_sb[64:128, pg, isl], o_un[0:64, 1, :], rb[:, 1, :])
                dq.append(norm)
                if pg == 1:
                    for i, (nt, ec) in enumerate(
                            (nt, ec) for nt in range(4 * ic, 4 * ic + 4)
                            for ec in range(2)):
                        dq.append(lambda nt=nt, ec=ec, i=i: emit_outproj(nt, ec, i % 2))
        while dq:
            dq.pop(0)()

    nc.compile()
    return nc


def _get_nc():
    if "nc" not in _cache:
        _cache["nc"] = _build()
    return _cache["nc"]


def _warr(w):
    """[E, ES] -> [128, KT*ES] sbuf layout: row p = concat_k w[k*128+p, :]."""
    return np.ascontiguousarray(
        w.reshape(KT, 128, ES).transpose(1, 0, 2).reshape(128, KT * ES)
    ).astype(BF16)


def _in_maps(query, kv, Wq, bq, Wkv, bkv, Wo, bo):
    strips = _estrips()
    qT = [np.ascontiguousarray(query[b].T).astype(BF16) for b in range(B)]
    kvT = [np.ascontiguousarray(kv[b].T).astype(BF16) for b in range(B)]
    maps = []
    for c in range(NCORES):
        b, g = c // 4, c % 4
        heads = _slot_heads(g)
        cols = np.concatenate([np.arange(h * D, (h + 1) * D) for h in heads])
        wo_arr = np.ascontiguousarray(
            Wo[cols, :].reshape(2, 128, E).transpose(1, 0, 2).reshape(128, 2 * E)
        ).astype(BF16)
        es_arr = np.ascontiguousarray(
            strips[g].transpose(1, 0, 2).reshape(128, HPC * USTRIP))
        bq_c = np.asarray(bq)[cols].astype(np.float32)        # [256]
        bk_c = np.asarray(bkv)[:E][cols].astype(np.float32)
        bv_c = np.asarray(bkv)[E:][cols].astype(BF16)
        bqk_arr = np.stack(
            [bq_c[0:128], bq_c[128:256], bk_c[0:128], bk_c[128:256]], axis=1)
        bvb_arr = np.ascontiguousarray(
            np.broadcast_to(bv_c[None, :], (128, ES)))
        maps.append({
            "qtt": qT[b],
            "kvt": kvT[b],
            "wq": _warr(Wq[:, cols]),
            "wk": _warr(Wkv[:, :E][:, cols]),
            "wv": _warr(Wkv[:, E:][:, cols]),
            "wo": wo_arr,
            "bqk": np.ascontiguousarray(bqk_arr),
            "bvb": bvb_arr,
            "estrip": es_arr,
        })
    return maps


def kernel(query, kv, Wq, bq, Wkv, bkv, Wo, bo, _collect=None):
    from concourse import bass_utils

    query = np.asarray(query, dtype=np.float32)
    kv = np.asarray(kv, dtype=np.float32)
    nc = _get_nc()
    maps = _in_maps(query, kv, np.asarray(Wq), np.asarray(bq), np.asarray(Wkv),
                    np.asarray(bkv), np.asarray(Wo), np.asarray(bo))
    res = bass_utils.run_bass_kernel_spmd(
        nc, maps, core_ids=list(range(NCORES)),
        **(_collect or {}),
    )
    if _collect is not None:
        _cache["last_results"] = res
    outp = np.zeros((B, N, E), dtype=np.float32)
    for c in range(NCORES):
        outp[c // 4] += res.results[c]["out"].astype(np.float32)
    outp += np.asarray(bo, dtype=np.float32)
    return outp
